# revision 34
# baseline (speedup 1.0000x reference)
"""Trainium2 Bass kernel for nn_Discriminator (DGCNN-style discriminator).

Sharding: data-parallel over batch. 16 point clouds -> 8 NeuronCores x 2.
No collectives; the host splits inputs and concatenates outputs.

Algorithm restructuring (exact, since lrelu is monotone and the 1x1 conv is
linear in the edge feature [x_j - x_i ; x_i]):
    edge_conv(x)[:, i] = lrelu( max_{j in knn(i)} u[:, j] + w[:, i] )
        u = W[:, :d] @ x            (per-point, no k dimension)
        w = (W[:, d:] - W[:, :d]) @ x
which removes the B*N*k*2d*out edge matmul entirely.

knn: top-20 of each row of P = x_i . x_j - ||x_j||^2/2 (same order as
-||x_i - x_j||^2 per row). Per 128-row tile: 3 rounds of DVE Max8
(max / max_index / match_replace). Neighbor gather via gpsimd dma_gather
(u rows from DRAM), max over the 20 gathered rows via strided DVE reduce.
"""

import numpy as np

B, N, KNN, NCORES = 16, 1024, 20, 8
BPC = B // NCORES  # batches per core
CONV_D = [6, 64, 64, 128]
CONV_O = [64, 64, 128, 256]
NEG = -1.0e30

_CACHE = {}


def _build_nc():
    import concourse.bacc as bacc
    import concourse.mybir as mybir
    import concourse.tile as tile
    from concourse.bass import IndirectOffsetOnAxis, ds, ts

    f32 = mybir.dt.float32
    f32r = mybir.dt.float32r
    f16 = mybir.dt.float16
    bf16 = mybir.dt.bfloat16
    u16 = mybir.dt.uint16
    i16 = mybir.dt.int16

    def rr(ap):
        return ap.bitcast(f32r)
    AF = mybir.ActivationFunctionType
    ALU = mybir.AluOpType
    AX = mybir.AxisListType.X

    nc = bacc.Bacc("TRN2", target_bir_lowering=False,
                   dynamic_dma_scratch_size=2**16)

    xt_d = nc.dram_tensor("xt", [BPC, 6, N], f32, kind="ExternalInput")
    y_d = nc.dram_tensor("y", [BPC, 16], f32, kind="ExternalInput")
    wstk_d = [
        nc.dram_tensor(f"wstk{c}", [CONV_D[c], 2 * CONV_O[c]], f32, kind="ExternalInput")
        for c in range(4)
    ]
    w4t_d = nc.dram_tensor("w4t", [512, 1024], f32, kind="ExternalInput")
    l0t_d = nc.dram_tensor("l0t", [1088, 512], f32, kind="ExternalInput")
    l1t_d = nc.dram_tensor("l1t", [512, 256], f32, kind="ExternalInput")
    l2t_d = nc.dram_tensor("l2t", [256, 1], f32, kind="ExternalInput")
    f0t_d = nc.dram_tensor("f0t", [16, 16], f32, kind="ExternalInput")
    f1t_d = nc.dram_tensor("f1t", [16, 64], f32, kind="ExternalInput")
    f0b_d = nc.dram_tensor("f0b", [16, 1], f32, kind="ExternalInput")
    f1b_d = nc.dram_tensor("f1b", [64, 1], f32, kind="ExternalInput")
    l2b_d = nc.dram_tensor("l2b", [1, 1], f32, kind="ExternalInput")
    id_d = nc.dram_tensor("ident", [128, 128], f32, kind="ExternalInput")
    out_d = nc.dram_tensor("out", [BPC, 1], f32, kind="ExternalOutput")

    # f-tile row sizes: f0, f1, f2, f3a, f3b — also the K-chunks of the W4 stage
    FSIZES = [64, 64, 128, 128, 128]

    with tile.TileContext(nc) as tc:
        with (
            tc.tile_pool(name="consts", bufs=1) as consts,
            tc.tile_pool(name="feat", bufs=1) as featp,
            tc.tile_pool(name="dram", bufs=2, space="DRAM") as dramp,
        ):
            ones_row = consts.tile([1, 128], f32, tag="ones")
            nc.vector.memset(ones_row, 1.0)
            ones_n = consts.tile([1, N], f32, tag="ones_n")
            nc.vector.memset(ones_n, 1.0)
            neghalf = consts.tile([128, 1], f32, tag="neghalf")
            nc.vector.memset(neghalf, -0.5)
            ident = consts.tile([128, 128], f32, tag="ident")
            nc.sync.dma_start(ident, id_d[:, :])
            wstk_sb = []
            for c in range(4):
                t = consts.tile([CONV_D[c], 2 * CONV_O[c]], f32, tag=f"wstk{c}", name=f"wstk{c}")
                nc.sync.dma_start(t, wstk_d[c][:, :])
                wstk_sb.append(t)

            # feature tiles (conv outputs, channel-major) per batch
            feat = {}
            for b in range(BPC):
                for fi, rows in enumerate(FSIZES):
                    feat[(b, fi)] = featp.tile([rows, N], f32, tag=f"f{b}_{fi}", name=f"f{b}_{fi}")
            xt0 = {}
            for b in range(BPC):
                xt0[b] = featp.tile([6, N], f32, tag=f"xt0_{b}", name=f"xt0_{b}")
                nc.sync.dma_start(xt0[b], xt_d[b])

            with (
                tc.tile_pool(name="sq", bufs=1) as sqp,
                tc.tile_pool(name="pw", bufs=2) as pwp,
                tc.tile_pool(name="mx", bufs=4) as mxp,
                tc.tile_pool(name="ix", bufs=3) as ixp,
                tc.tile_pool(name="uw", bufs=1) as uwp,
                tc.tile_pool(name="wcm", bufs=2) as wcmp,
                tc.tile_pool(name="idxw", bufs=4) as idxwp,
                tc.tile_pool(name="G", bufs=2) as gp,
                tc.tile_pool(name="m", bufs=1) as mp,
                tc.tile_pool(name="ps2", bufs=2, space="PSUM") as ps2,
                tc.tile_pool(name="ps1", bufs=4, space="PSUM") as ps1,
            ):
                for cv in range(4):
                    stash = {}
                    for b in range(BPC):
                        d, o = CONV_D[cv], CONV_O[cv]
                        if cv == 0:
                            xin = xt0[b][:, :]
                        else:
                            xin = feat[(b, cv - 1)][:, :]

                        # ---- -0.5*||x_j||^2 row [1, N] ----
                        sq = sqp.tile([128, N], f32, tag="sq")
                        nc.scalar.activation(sq[:d], xin, AF.Square)
                        nxp = ps2.tile([1, 2, 512], f32, tag="ps2")
                        for h in range(2):
                            nc.tensor.matmul(
                                nxp[:, h], neghalf[:d], sq[:d, ds(h * 512, 512)],
                                start=True, stop=True,
                            )
                        nxx = sqp.tile([1, N], f32, tag="nxx")
                        nc.scalar.copy(nxx, nxp.rearrange("p a b -> p (a b)"))
                        if d <= 126:
                            auglhs = sqp.tile([128, N], f32, tag="auglhs")
                            augrhs = sqp.tile([128, N], f32, tag="augrhs")
                            nc.scalar.copy(auglhs[:d], xin)
                            nc.sync.dma_start(auglhs[ds(d, 1)], ones_n[:, :])
                            nc.scalar.copy(augrhs[:d], xin)
                            nc.sync.dma_start(augrhs[ds(d, 1)], nxx[:, :])

                        # ---- u rows (fp16, padded to o16) -> DRAM for the
                        # transposed gather; w channel-major in fp32 ----
                        o16 = max(o, 128)
                        uw = uwp.tile([128, 8, o16], f16, tag="uw")
                        if o < o16:
                            nc.vector.memset(uw[:, :, ds(o, o16 - o)], 0.0)
                        for mm in range(8):
                            up = ps1.tile([128, o], f32, tag="ps1")
                            nc.tensor.matmul(
                                up, xin[:, ts(mm, 128)], wstk_sb[cv][:, :o],
                                start=True, stop=True,
                            )
                            nc.scalar.copy(uw[:, mm, :o], up)
                        ud = dramp.tile([N, o16], f16, tag=f"u{cv}", name=f"ud{cv}")
                        nc.sync.dma_start(
                            ud.rearrange("(mm q) e -> q mm e", q=128),
                            uw[:, :, :],
                        )
                        wcm = []
                        for j2 in range(max(1, o // 128)):
                            ow = min(128, o)
                            wt = wcmp.tile([128, N], f32, tag=f"wcm{j2}",
                                           name=f"wcm{j2}")
                            for h in range(2):
                                wp = ps1.tile([128, 512], f32, tag="ps1")
                                nc.tensor.matmul(
                                    wp[:ow], wstk_sb[cv][:, ds(o + j2 * 128, ow)],
                                    xin[:, ds(h * 512, 512)],
                                    start=True, stop=True,
                                )
                                nc.scalar.copy(wt[:ow, ds(h * 512, 512)], wp[:ow])
                            wcm.append(wt)

                        # ---- pairwise + top-24 per 128-row chunk ----
                        ixall = ixp.tile([128, 8, 24], u16, tag="ix")
                        for cc in range(8):
                            pp = ps2.tile([128, 2, 512], f32, tag="ps2")
                            for h in range(2):
                                if d <= 126:
                                    nc.tensor.matmul(
                                        pp[:, h], auglhs[:d + 1, ts(cc, 128)],
                                        augrhs[:d + 1, ds(h * 512, 512)],
                                        start=True, stop=True,
                                    )
                                else:
                                    nc.tensor.matmul(
                                        pp[:, h], xin[:, ts(cc, 128)],
                                        xin[:, ds(h * 512, 512)],
                                        start=True, stop=False,
                                    )
                                    nc.tensor.matmul(
                                        pp[:, h], ones_row, nxx[:, ds(h * 512, 512)],
                                        start=False, stop=True,
                                    )
                            pw = pwp.tile([128, N], f32, tag="pw")
                            nc.scalar.copy(pw, pp.rearrange("p a b -> p (a b)"))
                            mx = mxp.tile([128, 24], f32, tag="mx")
                            for r in range(3):
                                nc.vector.max(mx[:, ds(8 * r, 8)], pw)
                                nc.vector.max_index(
                                    ixall[:, cc, ds(8 * r, 8)], mx[:, ds(8 * r, 8)], pw
                                )
                                if r < 2:
                                    nc.vector.match_replace(
                                        pw, in_to_replace=mx[:, ds(8 * r, 8)],
                                        in_values=pw, imm_value=NEG,
                                    )

                        # ---- index tile for dma_gather (transpose mode):
                        # t-major list: position j = t*128 + q (q = 16*qh+ql)
                        # -> idx entry at partition j%16 = ql, col j//16 =
                        # t*8 + qh. One fused idx build per (conv, b):
                        # 8 qh-DMAs + 3 replication DMAs, on the Act queue.
                        idxw = idxwp.tile([128, 8, 192], i16, tag="idxw")
                        idxv = idxw.rearrange("p cc (t q8) -> p cc t q8", q8=8)
                        for qh in range(8):
                            nc.scalar.dma_start(
                                idxv[0:16, :, :, qh],
                                ixall.bitcast(i16)[ds(16 * qh, 16), :, :],
                            )
                        idxf = idxw.bitcast(f32)
                        nc.sync.dma_start(idxf[16:32], idxf[0:16])
                        nc.sync.dma_start(idxf[32:64], idxf[0:32])
                        nc.sync.dma_start(idxf[64:128], idxf[0:64])
                        stash[b] = (ud, idxw, wcm, d, o, o16)

                    # phase 2: gathers + neighbor-max trees, both batches.
                    # Keeping both batches' top-k ahead of the gathers in the
                    # in-order DVE stream hides gather latency.
                    for b in range(BPC):
                        ud, idxw, wcm, d, o, o16 = stash[b]
                        mcm = []
                        for ec in range(o16 // 128):
                            mt = mp.tile([128, N], f32, tag=f"mcm{ec}",
                                         name=f"mcm{ec}")
                            mcm.append(mt)
                        ne = o16 // 128
                        for blk in range(8):
                            G = gp.tile([128, ne, 2560], f16, tag="G")
                            nc.gpsimd.dma_gather(
                                G, ud[:, :], idxw[:, blk, :160],
                                2560, 2560, o16,
                                elem_step=o16, transpose=True,
                                single_packet=False,
                            )
                            # max over t=20 via f16 TT tree: 20->10->5,
                            # then 4(t0..t3)->2->1, fold t4 at the end.
                            tmp = mp.tile([128, ne, 2304], f16, tag="gtmp",
                                          name="gtmp")
                            for ec in range(ne):
                                Ge, Te = G[:, ec], tmp[:, ec]
                                nc.vector.tensor_tensor(
                                    Te[:, 0:1280], Ge[:, 0:1280],
                                    Ge[:, 1280:2560], op=ALU.max)
                                nc.vector.tensor_tensor(
                                    Te[:, 1280:1920], Te[:, 0:640],
                                    Te[:, 640:1280], op=ALU.max)
                                nc.vector.tensor_tensor(
                                    Te[:, 1920:2176], Te[:, 1280:1536],
                                    Te[:, 1536:1792], op=ALU.max)
                                nc.vector.tensor_tensor(
                                    Te[:, 2176:2304], Te[:, 1920:2048],
                                    Te[:, 2048:2176], op=ALU.max)
                                ow = min(128, o - 128 * ec) if o > 128 else o
                                nc.vector.tensor_tensor(
                                    mcm[ec][:ow, ts(blk, 128)],
                                    Te[:ow, 2176:2304], Te[:ow, 1792:1920],
                                    op=ALU.max)

                        # ---- f = lrelu(m + w) directly channel-major ----
                        for j2 in range(max(1, o // 128)):
                            ow = min(128, o)
                            if cv <= 1:
                                dstf = feat[(b, cv)]
                            elif cv == 2:
                                dstf = feat[(b, 2)]
                            else:
                                dstf = feat[(b, 3 + j2)]
                            nc.vector.tensor_add(
                                mcm[j2][:ow], mcm[j2][:ow], wcm[j2][:ow]
                            )
                            nc.vector.scalar_tensor_tensor(
                                dstf[:ow], mcm[j2][:ow], 0.2, mcm[j2][:ow],
                                op0=ALU.mult, op1=ALU.max,
                            )

            # ================= final stage =================
            with (
                tc.tile_pool(name="fin", bufs=1) as finp,
                tc.tile_pool(name="psh", bufs=2, space="PSUM") as psh,
                tc.tile_pool(name="psf", bufs=1, space="PSUM") as psf,
            ):
                w4t_sb = []
                row0 = 0
                for ki, rows in enumerate(FSIZES):
                    t = finp.tile([rows, N], f32, tag=f"w4t{ki}", name=f"w4t{ki}")
                    nc.sync.dma_start(t, w4t_d[ds(row0, rows)])
                    tb = finp.tile([rows, N], f16, tag=f"w4b{ki}", name=f"w4b{ki}")
                    nc.scalar.copy(tb, t)
                    w4t_sb.append(tb)
                    row0 += rows
                l0t_sb = finp.tile([128, 9, 512], f32, tag="l0t")
                for k in range(9):
                    rows = 128 if k < 8 else 64
                    nc.sync.dma_start(l0t_sb[:rows, k], l0t_d[ds(128 * k, rows)])
                l1t_sb = finp.tile([128, 4, 256], f32, tag="l1t")
                for k in range(4):
                    nc.sync.dma_start(l1t_sb[:, k], l1t_d[ds(128 * k, 128)])
                l2t_sb = finp.tile([128, 2, 1], f32, tag="l2t")
                for k in range(2):
                    nc.sync.dma_start(l2t_sb[:, k], l2t_d[ds(128 * k, 128)])
                f0t_sb = finp.tile([16, 16], f32, tag="f0t")
                nc.sync.dma_start(f0t_sb, f0t_d[:, :])
                f1t_sb = finp.tile([16, 64], f32, tag="f1t")
                nc.sync.dma_start(f1t_sb, f1t_d[:, :])
                f0b_sb = finp.tile([16, 1], f32, tag="f0b")
                nc.sync.dma_start(f0b_sb, f0b_d[:, :])
                f1b_sb = finp.tile([64, 1], f32, tag="f1b")
                nc.sync.dma_start(f1b_sb, f1b_d[:, :])
                l2b_sb = finp.tile([1, 1], f32, tag="l2b")
                nc.sync.dma_start(l2b_sb, l2b_d[:, :])
                ysb = finp.tile([16, BPC], f32, tag="ysb")
                for b in range(BPC):
                    nc.sync.dma_start(
                        ysb[:, ds(b, 1)], y_d[ds(b, 1)].rearrange("one p -> p one")
                    )
                res = finp.tile([1, BPC], f32, tag="res")

                # z0m[:, k, b]: k=0..7 -> g chunks (128 ch each); k=8 -> ye1|pad
                z0m = finp.tile([128, 9, BPC], f32, tag="z0m")
                nc.vector.memset(z0m[:, ds(8, 1)], 0.0)
                for b in range(BPC):
                    # h = W4 @ cat ; g = lrelu(max_n h)  (bf16 matmuls)
                    featb = []
                    for k in range(5):
                        fb = finp.tile([FSIZES[k], N], f16, tag=f"fb{k}",
                                       name=f"fb{b}_{k}")
                        nc.scalar.copy(fb, feat[(b, k)][:, :])
                        featb.append(fb)
                    gq = finp.tile([128, 8, 2], f32, tag=f"gq{b}")
                    for mt in range(8):
                        hp = psh.tile([128, 2, 512], f32, tag="hp")
                        for h2 in range(2):
                            for k in range(5):
                                nc.tensor.matmul(
                                    hp[:, h2],
                                    w4t_sb[k][:, ts(mt, 128)],
                                    featb[k][:, ds(h2 * 512, 512)],
                                    start=(k == 0), stop=(k == 4),
                                )
                        nc.vector.tensor_reduce(
                            gq[:, mt], hp, axis=AX, op=ALU.max,
                        )
                    g2 = finp.tile([128, 8], f32, tag=f"g2{b}")
                    nc.vector.tensor_reduce(g2, gq, axis=AX, op=ALU.max)
                    nc.vector.scalar_tensor_tensor(
                        z0m[:, 0:8, ds(b, 1)].rearrange("p a b -> p (a b)"),
                        g2, 0.2, g2, op0=ALU.mult, op1=ALU.max,
                    )

                # y-embedding head, both batches at once
                yp = psf.tile([128, BPC], f32, tag="ypp")
                nc.tensor.matmul(yp[:16], f0t_sb, ysb, start=True, stop=True)
                ye0 = finp.tile([16, BPC], f32, tag="ye0")
                yeb = finp.tile([16, BPC], f32, tag="yeb")
                nc.scalar.activation(yeb, yp[:16], AF.Identity, bias=f0b_sb)
                nc.vector.scalar_tensor_tensor(
                    ye0, yeb, 0.2, yeb, op0=ALU.mult, op1=ALU.max)
                yp2 = psf.tile([128, BPC], f32, tag="ypp")
                nc.tensor.matmul(yp2[:64], f1t_sb, ye0, start=True, stop=True)
                ye1b = finp.tile([64, BPC], f32, tag="ye1b")
                nc.scalar.activation(ye1b, yp2[:64], AF.Identity, bias=f1b_sb)
                nc.vector.scalar_tensor_tensor(
                    z0m[0:64, 8], ye1b, 0.2, ye1b, op0=ALU.mult, op1=ALU.max)

                # z = lrelu(L0 z); z = lrelu(L1 z); out = L2 z + b  (batched)
                z1p = psf.tile([128, 4, BPC], f32, tag="ypp")
                for mt in range(4):
                    for k in range(9):
                        rows = 128 if k < 8 else 64
                        nc.tensor.matmul(
                            z1p[:, mt],
                            l0t_sb[:rows, k, ts(mt, 128)],
                            z0m[:rows, k],
                            start=(k == 0), stop=(k == 8),
                        )
                z1 = finp.tile([128, 4, BPC], f32, tag="z1")
                z1c = finp.tile([128, 4, BPC], f32, tag="z1c")
                nc.scalar.copy(z1c, z1p)
                nc.vector.scalar_tensor_tensor(
                    z1, z1c, 0.2, z1c, op0=ALU.mult, op1=ALU.max)
                z2p = psf.tile([128, 2, BPC], f32, tag="ypp")
                for mt in range(2):
                    for k in range(4):
                        nc.tensor.matmul(
                            z2p[:, mt],
                            l1t_sb[:, k, ts(mt, 128)],
                            z1[:, k],
                            start=(k == 0), stop=(k == 3),
                        )
                z2 = finp.tile([128, 2, BPC], f32, tag="z2")
                z2c = finp.tile([128, 2, BPC], f32, tag="z2c")
                nc.scalar.copy(z2c, z2p)
                nc.vector.scalar_tensor_tensor(
                    z2, z2c, 0.2, z2c, op0=ALU.mult, op1=ALU.max)
                zp = psf.tile([1, BPC], f32, tag="ypp")
                for k in range(2):
                    nc.tensor.matmul(
                        zp, l2t_sb[:, k], z2[:, k],
                        start=(k == 0), stop=(k == 1),
                    )
                nc.scalar.activation(res, zp, AF.Identity, bias=l2b_sb)
                nc.sync.dma_start(out_d.rearrange("b one -> one b"), res)

    nc.compile()
    return nc


def _get_nc():
    if "nc" not in _CACHE:
        _CACHE["nc"] = _build_nc()
    return _CACHE["nc"]


def make_in_maps(x, y, W0, W1, W2, W3, W4, L0, L1, L2_w, L2_b, F0_w, F0_b, F1_w, F1_b):
    def f32c(a):
        return np.ascontiguousarray(np.asarray(a, dtype=np.float32))

    x, y = f32c(x), f32c(y)
    xt = np.ascontiguousarray(np.swapaxes(x, 1, 2))  # [B, 6, N]

    def stk(W, d):
        W = f32c(W)
        w1p, w2p = W[:, :d], W[:, d:]
        return np.ascontiguousarray(
            np.concatenate([w1p.T, (w2p - w1p).T], axis=1)
        )

    base = {
        "wstk0": stk(W0, 6),
        "wstk1": stk(W1, 64),
        "wstk2": stk(W2, 64),
        "wstk3": stk(W3, 128),
        "w4t": np.ascontiguousarray(f32c(W4).T),
        "l0t": np.ascontiguousarray(f32c(L0).T),
        "l1t": np.ascontiguousarray(f32c(L1).T),
        "l2t": np.ascontiguousarray(f32c(L2_w).T),
        "f0t": np.ascontiguousarray(f32c(F0_w).T),
        "f1t": np.ascontiguousarray(f32c(F1_w).T),
        "f0b": f32c(F0_b).reshape(16, 1),
        "f1b": f32c(F1_b).reshape(64, 1),
        "l2b": f32c(L2_b).reshape(1, 1),
        "ident": np.eye(128, dtype=np.float32),
    }
    return [
        {**base, "xt": xt[c * BPC:(c + 1) * BPC], "y": y[c * BPC:(c + 1) * BPC]}
        for c in range(NCORES)
    ]


def kernel(**inputs):
    from concourse.bass_utils import run_bass_kernel_spmd

    nc = _get_nc()
    in_maps = make_in_maps(**inputs)
    res = run_bass_kernel_spmd(nc, in_maps, core_ids=list(range(NCORES)))
    return np.concatenate([r["out"] for r in res.results], axis=0)


if __name__ == "__main__":
    nc = _build_nc()
    print("built + compiled OK")



# revision 35
# speedup vs baseline: 1.0291x; 1.0291x over previous
"""Trainium2 Bass kernel for nn_Discriminator (DGCNN-style discriminator).

Sharding: data-parallel over batch. 16 point clouds -> 8 NeuronCores x 2.
No collectives; the host splits inputs and concatenates outputs.

Algorithm restructuring (exact, since lrelu is monotone and the 1x1 conv is
linear in the edge feature [x_j - x_i ; x_i]):
    edge_conv(x)[:, i] = lrelu( max_{j in knn(i)} u[:, j] + w[:, i] )
        u = W[:, :d] @ x            (per-point, no k dimension)
        w = (W[:, d:] - W[:, :d]) @ x
which removes the B*N*k*2d*out edge matmul entirely.

knn: top-20 of each row of P = x_i . x_j - ||x_j||^2/2 (same order as
-||x_i - x_j||^2 per row). Per 128-row tile: 3 rounds of DVE Max8
(max / max_index / match_replace). Neighbor gather via gpsimd dma_gather
(u rows from DRAM), max over the 20 gathered rows via strided DVE reduce.
"""

import numpy as np

B, N, KNN, NCORES = 16, 1024, 20, 8
BPC = B // NCORES  # batches per core
CONV_D = [6, 64, 64, 128]
CONV_O = [64, 64, 128, 256]
NEG = -1.0e30

_CACHE = {}


def _build_nc():
    import concourse.bacc as bacc
    import concourse.mybir as mybir
    import concourse.tile as tile
    from concourse.bass import IndirectOffsetOnAxis, ds, ts

    f32 = mybir.dt.float32
    f32r = mybir.dt.float32r
    f16 = mybir.dt.float16
    bf16 = mybir.dt.bfloat16
    u16 = mybir.dt.uint16
    i16 = mybir.dt.int16

    def rr(ap):
        return ap.bitcast(f32r)
    AF = mybir.ActivationFunctionType
    ALU = mybir.AluOpType
    AX = mybir.AxisListType.X

    nc = bacc.Bacc("TRN2", target_bir_lowering=False,
                   dynamic_dma_scratch_size=2**16)

    xt_d = nc.dram_tensor("xt", [BPC, 6, N], f32, kind="ExternalInput")
    y_d = nc.dram_tensor("y", [BPC, 16], f32, kind="ExternalInput")
    wstk_d = [
        nc.dram_tensor(f"wstk{c}", [CONV_D[c], 2 * CONV_O[c]], f32, kind="ExternalInput")
        for c in range(4)
    ]
    w4t_d = nc.dram_tensor("w4t", [512, 1024], f32, kind="ExternalInput")
    l0t_d = nc.dram_tensor("l0t", [1088, 512], f32, kind="ExternalInput")
    l1t_d = nc.dram_tensor("l1t", [512, 256], f32, kind="ExternalInput")
    l2t_d = nc.dram_tensor("l2t", [256, 1], f32, kind="ExternalInput")
    f0t_d = nc.dram_tensor("f0t", [16, 16], f32, kind="ExternalInput")
    f1t_d = nc.dram_tensor("f1t", [16, 64], f32, kind="ExternalInput")
    f0b_d = nc.dram_tensor("f0b", [16, 1], f32, kind="ExternalInput")
    f1b_d = nc.dram_tensor("f1b", [64, 1], f32, kind="ExternalInput")
    l2b_d = nc.dram_tensor("l2b", [1, 1], f32, kind="ExternalInput")
    id_d = nc.dram_tensor("ident", [128, 128], f32, kind="ExternalInput")
    out_d = nc.dram_tensor("out", [BPC, 1], f32, kind="ExternalOutput")

    # f-tile row sizes: f0, f1, f2, f3a, f3b — also the K-chunks of the W4 stage
    FSIZES = [64, 64, 128, 128, 128]

    with tile.TileContext(nc) as tc:
        with (
            tc.tile_pool(name="consts", bufs=1) as consts,
            tc.tile_pool(name="feat", bufs=1) as featp,
            tc.tile_pool(name="dram", bufs=2, space="DRAM") as dramp,
        ):
            ones_row = consts.tile([1, 128], f32, tag="ones")
            nc.vector.memset(ones_row, 1.0)
            ones_n = consts.tile([1, N], f32, tag="ones_n")
            nc.vector.memset(ones_n, 1.0)
            neghalf = consts.tile([128, 1], f32, tag="neghalf")
            nc.vector.memset(neghalf, -0.5)
            ident = consts.tile([128, 128], f32, tag="ident")
            nc.sync.dma_start(ident, id_d[:, :])
            wstk_sb = []
            for c in range(4):
                t = consts.tile([CONV_D[c], 2 * CONV_O[c]], f32, tag=f"wstk{c}", name=f"wstk{c}")
                nc.sync.dma_start(t, wstk_d[c][:, :])
                wstk_sb.append(t)

            # feature tiles (conv outputs, channel-major) per batch
            feat = {}
            for b in range(BPC):
                for fi, rows in enumerate(FSIZES):
                    feat[(b, fi)] = featp.tile([rows, N], f32, tag=f"f{b}_{fi}", name=f"f{b}_{fi}")
            xt0 = {}
            for b in range(BPC):
                xt0[b] = featp.tile([6, N], f32, tag=f"xt0_{b}", name=f"xt0_{b}")
                nc.sync.dma_start(xt0[b], xt_d[b])

            with (
                tc.tile_pool(name="sq", bufs=1) as sqp,
                tc.tile_pool(name="pw", bufs=2) as pwp,
                tc.tile_pool(name="mx", bufs=4) as mxp,
                tc.tile_pool(name="ix", bufs=3) as ixp,
                tc.tile_pool(name="uw", bufs=1) as uwp,
                tc.tile_pool(name="wcm", bufs=1) as wcmp,
                tc.tile_pool(name="idxw", bufs=4) as idxwp,
                tc.tile_pool(name="G", bufs=2) as gp,
                tc.tile_pool(name="m", bufs=1) as mp,
                tc.tile_pool(name="ps2", bufs=2, space="PSUM") as ps2,
                tc.tile_pool(name="ps1", bufs=4, space="PSUM") as ps1,
            ):
                for cv in range(4):
                    for b in range(BPC):
                        d, o = CONV_D[cv], CONV_O[cv]
                        if cv == 0:
                            xin = xt0[b][:, :]
                        else:
                            xin = feat[(b, cv - 1)][:, :]

                        # ---- -0.5*||x_j||^2 row [1, N] ----
                        sq = sqp.tile([128, N], f32, tag="sq")
                        nc.scalar.activation(sq[:d], xin, AF.Square)
                        nxp = ps2.tile([1, 2, 512], f32, tag="ps2")
                        for h in range(2):
                            nc.tensor.matmul(
                                nxp[:, h], neghalf[:d], sq[:d, ds(h * 512, 512)],
                                start=True, stop=True,
                            )
                        nxx = sqp.tile([1, N], f32, tag="nxx")
                        nc.scalar.copy(nxx, nxp.rearrange("p a b -> p (a b)"))
                        if d <= 126:
                            auglhs = sqp.tile([128, N], f32, tag="auglhs")
                            augrhs = sqp.tile([128, N], f32, tag="augrhs")
                            nc.scalar.copy(auglhs[:d], xin)
                            nc.sync.dma_start(auglhs[ds(d, 1)], ones_n[:, :])
                            nc.scalar.copy(augrhs[:d], xin)
                            nc.sync.dma_start(augrhs[ds(d, 1)], nxx[:, :])

                        # ---- u rows (fp16, padded to o16) -> DRAM for the
                        # transposed gather; w channel-major in fp32 ----
                        o16 = max(o, 128)
                        uw = uwp.tile([128, 8, o16], f16, tag="uw")
                        if o < o16:
                            nc.vector.memset(uw[:, :, ds(o, o16 - o)], 0.0)
                        for mm in range(8):
                            up = ps1.tile([128, o], f32, tag="ps1")
                            nc.tensor.matmul(
                                up, xin[:, ts(mm, 128)], wstk_sb[cv][:, :o],
                                start=True, stop=True,
                            )
                            nc.scalar.copy(uw[:, mm, :o], up)
                        ud = dramp.tile([N, o16], f16, tag=f"u{cv}", name=f"ud{cv}")
                        nc.sync.dma_start(
                            ud.rearrange("(mm q) e -> q mm e", q=128),
                            uw[:, :, :],
                        )
                        wcm = []
                        for j2 in range(max(1, o // 128)):
                            ow = min(128, o)
                            wt = wcmp.tile([128, N], f32, tag=f"wcm{j2}",
                                           name=f"wcm{j2}")
                            for h in range(2):
                                wp = ps1.tile([128, 512], f32, tag="ps1")
                                nc.tensor.matmul(
                                    wp[:ow], wstk_sb[cv][:, ds(o + j2 * 128, ow)],
                                    xin[:, ds(h * 512, 512)],
                                    start=True, stop=True,
                                )
                                nc.scalar.copy(wt[:ow, ds(h * 512, 512)], wp[:ow])
                            wcm.append(wt)

                        # ---- pairwise + top-24 per 128-row chunk ----
                        ixall = ixp.tile([128, 8, 24], u16, tag="ix")
                        for cc in range(8):
                            pp = ps2.tile([128, 2, 512], f32, tag="ps2")
                            for h in range(2):
                                if d <= 126:
                                    nc.tensor.matmul(
                                        pp[:, h], auglhs[:d + 1, ts(cc, 128)],
                                        augrhs[:d + 1, ds(h * 512, 512)],
                                        start=True, stop=True,
                                    )
                                else:
                                    nc.tensor.matmul(
                                        pp[:, h], xin[:, ts(cc, 128)],
                                        xin[:, ds(h * 512, 512)],
                                        start=True, stop=False,
                                    )
                                    nc.tensor.matmul(
                                        pp[:, h], ones_row, nxx[:, ds(h * 512, 512)],
                                        start=False, stop=True,
                                    )
                            pw = pwp.tile([128, N], f32, tag="pw")
                            nc.scalar.copy(pw, pp.rearrange("p a b -> p (a b)"))
                            mx = mxp.tile([128, 24], f32, tag="mx")
                            for r in range(3):
                                nc.vector.max(mx[:, ds(8 * r, 8)], pw)
                                nc.vector.max_index(
                                    ixall[:, cc, ds(8 * r, 8)], mx[:, ds(8 * r, 8)], pw
                                )
                                if r < 2:
                                    nc.vector.match_replace(
                                        pw, in_to_replace=mx[:, ds(8 * r, 8)],
                                        in_values=pw, imm_value=NEG,
                                    )

                        # ---- index tile for dma_gather (transpose mode):
                        # t-major list: position j = t*128 + q (q = 16*qh+ql)
                        # -> idx entry at partition j%16 = ql, col j//16 =
                        # t*8 + qh. One fused idx build per (conv, b):
                        # 8 qh-DMAs + 3 replication DMAs, on the Act queue.
                        mcm = []
                        for ec in range(o16 // 128):
                            mt = mp.tile([128, N], f32, tag=f"mcm{ec}",
                                         name=f"mcm{ec}")
                            mcm.append(mt)
                        idxw = idxwp.tile([128, 8, 192], i16, tag="idxw")
                        idxv = idxw.rearrange("p cc (t q8) -> p cc t q8", q8=8)
                        for qh in range(8):
                            nc.scalar.dma_start(
                                idxv[0:16, :, :, qh],
                                ixall.bitcast(i16)[ds(16 * qh, 16), :, :],
                            )
                        idxf = idxw.bitcast(f32)
                        nc.sync.dma_start(idxf[16:32], idxf[0:16])
                        nc.sync.dma_start(idxf[32:64], idxf[0:32])
                        nc.sync.dma_start(idxf[64:128], idxf[0:64])
                        ne = o16 // 128
                        for blk in range(8):
                            G = gp.tile([128, ne, 2560], f16, tag="G")
                            nc.gpsimd.dma_gather(
                                G, ud[:, :], idxw[:, blk, :160],
                                2560, 2560, o16,
                                elem_step=o16, transpose=True,
                                single_packet=False,
                            )
                            # max over t=20 via f16 TT tree: 20->10->5,
                            # then 4(t0..t3)->2->1, fold t4 at the end.
                            tmp = mp.tile([128, ne, 2304], f16, tag="gtmp",
                                          name="gtmp")
                            for ec in range(ne):
                                Ge, Te = G[:, ec], tmp[:, ec]
                                nc.vector.tensor_tensor(
                                    Te[:, 0:1280], Ge[:, 0:1280],
                                    Ge[:, 1280:2560], op=ALU.max)
                                nc.vector.tensor_tensor(
                                    Te[:, 1280:1920], Te[:, 0:640],
                                    Te[:, 640:1280], op=ALU.max)
                                nc.vector.tensor_tensor(
                                    Te[:, 1920:2176], Te[:, 1280:1536],
                                    Te[:, 1536:1792], op=ALU.max)
                                nc.vector.tensor_tensor(
                                    Te[:, 2176:2304], Te[:, 1920:2048],
                                    Te[:, 2048:2176], op=ALU.max)
                                ow = min(128, o - 128 * ec) if o > 128 else o
                                nc.vector.tensor_tensor(
                                    mcm[ec][:ow, ts(blk, 128)],
                                    Te[:ow, 2176:2304], Te[:ow, 1792:1920],
                                    op=ALU.max)

                        # ---- f = lrelu(m + w) directly channel-major ----
                        for j2 in range(max(1, o // 128)):
                            ow = min(128, o)
                            if cv <= 1:
                                dstf = feat[(b, cv)]
                            elif cv == 2:
                                dstf = feat[(b, 2)]
                            else:
                                dstf = feat[(b, 3 + j2)]
                            nc.vector.tensor_add(
                                mcm[j2][:ow], mcm[j2][:ow], wcm[j2][:ow]
                            )
                            nc.vector.scalar_tensor_tensor(
                                dstf[:ow], mcm[j2][:ow], 0.2, mcm[j2][:ow],
                                op0=ALU.mult, op1=ALU.max,
                            )

            # ================= final stage =================
            with (
                tc.tile_pool(name="fin", bufs=1) as finp,
                tc.tile_pool(name="psh", bufs=2, space="PSUM") as psh,
                tc.tile_pool(name="psf", bufs=1, space="PSUM") as psf,
            ):
                w4t_sb = []
                row0 = 0
                for ki, rows in enumerate(FSIZES):
                    t = finp.tile([rows, N], f32, tag=f"w4t{ki}", name=f"w4t{ki}")
                    nc.sync.dma_start(t, w4t_d[ds(row0, rows)])
                    tb = finp.tile([rows, N], f16, tag=f"w4b{ki}", name=f"w4b{ki}")
                    nc.scalar.copy(tb, t)
                    w4t_sb.append(tb)
                    row0 += rows
                l0t_sb = finp.tile([128, 9, 512], f32, tag="l0t")
                for k in range(9):
                    rows = 128 if k < 8 else 64
                    nc.sync.dma_start(l0t_sb[:rows, k], l0t_d[ds(128 * k, rows)])
                l1t_sb = finp.tile([128, 4, 256], f32, tag="l1t")
                for k in range(4):
                    nc.sync.dma_start(l1t_sb[:, k], l1t_d[ds(128 * k, 128)])
                l2t_sb = finp.tile([128, 2, 1], f32, tag="l2t")
                for k in range(2):
                    nc.sync.dma_start(l2t_sb[:, k], l2t_d[ds(128 * k, 128)])
                f0t_sb = finp.tile([16, 16], f32, tag="f0t")
                nc.sync.dma_start(f0t_sb, f0t_d[:, :])
                f1t_sb = finp.tile([16, 64], f32, tag="f1t")
                nc.sync.dma_start(f1t_sb, f1t_d[:, :])
                f0b_sb = finp.tile([16, 1], f32, tag="f0b")
                nc.sync.dma_start(f0b_sb, f0b_d[:, :])
                f1b_sb = finp.tile([64, 1], f32, tag="f1b")
                nc.sync.dma_start(f1b_sb, f1b_d[:, :])
                l2b_sb = finp.tile([1, 1], f32, tag="l2b")
                nc.sync.dma_start(l2b_sb, l2b_d[:, :])
                ysb = finp.tile([16, BPC], f32, tag="ysb")
                for b in range(BPC):
                    nc.sync.dma_start(
                        ysb[:, ds(b, 1)], y_d[ds(b, 1)].rearrange("one p -> p one")
                    )
                res = finp.tile([1, BPC], f32, tag="res")

                # z0m[:, k, b]: k=0..7 -> g chunks (128 ch each); k=8 -> ye1|pad
                z0m = finp.tile([128, 9, BPC], f32, tag="z0m")
                nc.vector.memset(z0m[:, ds(8, 1)], 0.0)
                for b in range(BPC):
                    # h = W4 @ cat ; g = lrelu(max_n h)  (bf16 matmuls)
                    featb = []
                    for k in range(5):
                        fb = finp.tile([FSIZES[k], N], f16, tag=f"fb{k}",
                                       name=f"fb{b}_{k}")
                        nc.scalar.copy(fb, feat[(b, k)][:, :])
                        featb.append(fb)
                    gq = finp.tile([128, 8, 2], f32, tag=f"gq{b}")
                    for mt in range(8):
                        hp = psh.tile([128, 2, 512], f32, tag="hp")
                        for h2 in range(2):
                            for k in range(5):
                                nc.tensor.matmul(
                                    hp[:, h2],
                                    w4t_sb[k][:, ts(mt, 128)],
                                    featb[k][:, ds(h2 * 512, 512)],
                                    start=(k == 0), stop=(k == 4),
                                )
                        nc.vector.tensor_reduce(
                            gq[:, mt], hp, axis=AX, op=ALU.max,
                        )
                    g2 = finp.tile([128, 8], f32, tag=f"g2{b}")
                    nc.vector.tensor_reduce(g2, gq, axis=AX, op=ALU.max)
                    nc.vector.scalar_tensor_tensor(
                        z0m[:, 0:8, ds(b, 1)].rearrange("p a b -> p (a b)"),
                        g2, 0.2, g2, op0=ALU.mult, op1=ALU.max,
                    )

                # y-embedding head, both batches at once
                yp = psf.tile([128, BPC], f32, tag="ypp")
                nc.tensor.matmul(yp[:16], f0t_sb, ysb, start=True, stop=True)
                ye0 = finp.tile([16, BPC], f32, tag="ye0")
                yeb = finp.tile([16, BPC], f32, tag="yeb")
                nc.scalar.activation(yeb, yp[:16], AF.Identity, bias=f0b_sb)
                nc.vector.scalar_tensor_tensor(
                    ye0, yeb, 0.2, yeb, op0=ALU.mult, op1=ALU.max)
                yp2 = psf.tile([128, BPC], f32, tag="ypp")
                nc.tensor.matmul(yp2[:64], f1t_sb, ye0, start=True, stop=True)
                ye1b = finp.tile([64, BPC], f32, tag="ye1b")
                nc.scalar.activation(ye1b, yp2[:64], AF.Identity, bias=f1b_sb)
                nc.vector.scalar_tensor_tensor(
                    z0m[0:64, 8], ye1b, 0.2, ye1b, op0=ALU.mult, op1=ALU.max)

                # z = lrelu(L0 z); z = lrelu(L1 z); out = L2 z + b  (batched)
                z1p = psf.tile([128, 4, BPC], f32, tag="ypp")
                for mt in range(4):
                    for k in range(9):
                        rows = 128 if k < 8 else 64
                        nc.tensor.matmul(
                            z1p[:, mt],
                            l0t_sb[:rows, k, ts(mt, 128)],
                            z0m[:rows, k],
                            start=(k == 0), stop=(k == 8),
                        )
                z1 = finp.tile([128, 4, BPC], f32, tag="z1")
                z1c = finp.tile([128, 4, BPC], f32, tag="z1c")
                nc.scalar.copy(z1c, z1p)
                nc.vector.scalar_tensor_tensor(
                    z1, z1c, 0.2, z1c, op0=ALU.mult, op1=ALU.max)
                z2p = psf.tile([128, 2, BPC], f32, tag="ypp")
                for mt in range(2):
                    for k in range(4):
                        nc.tensor.matmul(
                            z2p[:, mt],
                            l1t_sb[:, k, ts(mt, 128)],
                            z1[:, k],
                            start=(k == 0), stop=(k == 3),
                        )
                z2 = finp.tile([128, 2, BPC], f32, tag="z2")
                z2c = finp.tile([128, 2, BPC], f32, tag="z2c")
                nc.scalar.copy(z2c, z2p)
                nc.vector.scalar_tensor_tensor(
                    z2, z2c, 0.2, z2c, op0=ALU.mult, op1=ALU.max)
                zp = psf.tile([1, BPC], f32, tag="ypp")
                for k in range(2):
                    nc.tensor.matmul(
                        zp, l2t_sb[:, k], z2[:, k],
                        start=(k == 0), stop=(k == 1),
                    )
                nc.scalar.activation(res, zp, AF.Identity, bias=l2b_sb)
                nc.sync.dma_start(out_d.rearrange("b one -> one b"), res)

    nc.compile()
    return nc


def _get_nc():
    if "nc" not in _CACHE:
        _CACHE["nc"] = _build_nc()
    return _CACHE["nc"]


def make_in_maps(x, y, W0, W1, W2, W3, W4, L0, L1, L2_w, L2_b, F0_w, F0_b, F1_w, F1_b):
    def f32c(a):
        return np.ascontiguousarray(np.asarray(a, dtype=np.float32))

    x, y = f32c(x), f32c(y)
    xt = np.ascontiguousarray(np.swapaxes(x, 1, 2))  # [B, 6, N]

    def stk(W, d):
        W = f32c(W)
        w1p, w2p = W[:, :d], W[:, d:]
        return np.ascontiguousarray(
            np.concatenate([w1p.T, (w2p - w1p).T], axis=1)
        )

    base = {
        "wstk0": stk(W0, 6),
        "wstk1": stk(W1, 64),
        "wstk2": stk(W2, 64),
        "wstk3": stk(W3, 128),
        "w4t": np.ascontiguousarray(f32c(W4).T),
        "l0t": np.ascontiguousarray(f32c(L0).T),
        "l1t": np.ascontiguousarray(f32c(L1).T),
        "l2t": np.ascontiguousarray(f32c(L2_w).T),
        "f0t": np.ascontiguousarray(f32c(F0_w).T),
        "f1t": np.ascontiguousarray(f32c(F1_w).T),
        "f0b": f32c(F0_b).reshape(16, 1),
        "f1b": f32c(F1_b).reshape(64, 1),
        "l2b": f32c(L2_b).reshape(1, 1),
        "ident": np.eye(128, dtype=np.float32),
    }
    return [
        {**base, "xt": xt[c * BPC:(c + 1) * BPC], "y": y[c * BPC:(c + 1) * BPC]}
        for c in range(NCORES)
    ]


def kernel(**inputs):
    from concourse.bass_utils import run_bass_kernel_spmd

    nc = _get_nc()
    in_maps = make_in_maps(**inputs)
    res = run_bass_kernel_spmd(nc, in_maps, core_ids=list(range(NCORES)))
    return np.concatenate([r["out"] for r in res.results], axis=0)


if __name__ == "__main__":
    nc = _build_nc()
    print("built + compiled OK")

